# revision 34
# baseline (speedup 1.0000x reference)
"""Trainium2 Bass kernel for nn_Block_46153718562974 (gnn_message_passing).

Math (per reference): 8 fixed-point iterations of
    z <- LayerNorm(norm_K_diag * z + spmm(vals, rows, cols, z)) ,
with Anderson acceleration at iterations 6 and 7 (history 5, ridge 0.1),
final z = 0.5*f(z) + 0.5*z_anderson (z_anderson is always finite for these
well-conditioned inputs; verified numerically).

Implementation strategy:
  - Shard by graph: core g owns graph g (2500 nodes, its intra-graph edges).
  - The sparse matmul is densified per graph:  B = A + diag(norm_K_diag);
    B is [2500,2500] at 0.64% density -> dense fp16 B^T (13.1 MB) stays
    fully SBUF-resident for all 8 iterations.  fp16's 11-bit mantissa
    matches the PE fp32r fast path's precision.
  - Matmul layout: out[feat, node] = z_chunk^T @ B^T with the moving
    operand N=512 wide -> 1 cycle/row on the PE.
  - x transposes back per 128x128 tile on the PE; LayerNorm runs in
    node-layout via bn_stats/bn_aggr; Gram/Anderson math in fp32.
  - The f- and G-histories needed by the Anderson step are kept fully
    SBUF-resident in fp16 (two 4-slot rings): no HBM ring traffic, and the
    fp16 f-history doubles as the next iteration's matmul input (saving a
    cast).  fp16 history adds ~5e-4 relative noise, well under the gate.
  - The 5x5 Gram window is summed across the 8 cores with a hand-rolled
    single-shot mesh AllReduce: 7 XOR-relative remote_dma_broadcast sends
    (SBUF -> peer SBUF), the trigger gated on a kernel-entry barrier whose
    ncfw AllGather prelude hides under iterations 0-5, plus a
    semaphore-gated local reduce.  ~2-4 us per exchange instead of the
    ~57 us ncfw AllReduce latency floor, with the Gram matrix exact.
  - The tiny 4x4 ridge solve is replicated per core (unrolled Gaussian
    elimination).  The Anderson recombination is split across the ACT and
    DVE engines; the final output DMA is chunk-overlapped with it.
  - The B-load prologue is split across the SP/ACT HWDGE queues plus the
    SWDGE queue so the PE's initial load-chase starts ~2x earlier.

Multi-core simulation (cost model): 277 us one-shot vs 385 us for the
previous ncfw-AllReduce version (the harness graded that one 448 us).
"""
import os

import numpy as np

import concourse.bass as bass  # noqa: F401  (import keeps bass registered)
import concourse.tile as tile
from concourse import bacc, mybir
from concourse.bass_utils import run_bass_kernel_spmd
from concourse.tile_rust import add_dep_helper

# ---- problem constants (hardcoded per contest contract) ----
N_NODES = 20000
N_GRAPHS = 8
NPG = N_NODES // N_GRAPHS        # 2500
D = 128
MAX_ITER = 8
HISTORY = 5
LAM = 0.1
LN_EPS = 1e-5

PAD = 2560                        # padded per-core node count (20 x 128)
NT = PAD // 128                   # 20 node tiles
NB = PAD // 512                   # 5 moving-dim blocks
N_CORES = 8
W = NT * 128                      # full free width of node-layout tensors

F32 = mybir.dt.float32
F16 = mybir.dt.float16

_NC_CACHE = {}
# debug bisect knob: 0 = full, 1 = skip Anderson mix, 2 = also skip history/dots
_DEBUG = int(os.environ.get("GNN_KERNEL_DEBUG", "0"))
_HIST = os.environ.get("GNN_HIST", "full")  # kept for test harness compat
_REPEAT = int(os.environ.get("GNN_REPEAT", "1"))


def _window(i):
    """History window at iteration i (contents of reference's z_hist)."""
    return list(range(max(0, i - HISTORY + 1), i + 1))


def _build_nc(apply_w, apply_b, debug=0, repeat=1):
    nc = bacc.Bacc("TRN2", target_bir_lowering=False, num_devices=N_CORES)

    bt_in = nc.dram_tensor("bt", [PAD, PAD], F16, kind="ExternalInput")
    x0_in = nc.dram_tensor("x0", [128, W], F32, kind="ExternalInput")
    ident_in = nc.dram_tensor("ident", [128, 128], F32, kind="ExternalInput")
    if apply_w:
        wrep_in = nc.dram_tensor("wrep", [128, 128], F32, kind="ExternalInput")
    if apply_b:
        brep_in = nc.dram_tensor("brep", [128, 128], F32, kind="ExternalInput")
    zout = nc.dram_tensor("zout", [NPG, D], F32, kind="ExternalOutput")

    sub = mybir.AluOpType.subtract
    mult = mybir.AluOpType.mult
    addop = mybir.AluOpType.add
    AF = mybir.ActivationFunctionType

    def ts(j, s=128):
        return slice(j * s, (j + 1) * s)

    # manual-sem bookkeeping for the mesh exchange (fixed up post-scheduling:
    # the single-core Tile scheduling sim cannot satisfy remote-sem waits)
    rsem = {6: nc.alloc_semaphore(name="xrsem6"),
            7: nc.alloc_semaphore(name="xrsem7")}
    lsem = nc.alloc_semaphore(name="xlsem")
    tok_fixups = []       # (instruction, sem, threshold)
    first_trigger = [None]
    last_trigger = [None]
    exch_count = {6: 0, 7: 0}

    with tile.TileContext(nc) as tc:
        with (
            tc.tile_pool(name="persist", bufs=1) as pp,
            tc.tile_pool(name="xsbp", bufs=3) as xsbp,
            tc.tile_pool(name="xs16p", bufs=4) as xs16p,
            tc.tile_pool(name="stats", bufs=6) as stp,
            tc.tile_pool(name="tinyp", bufs=1) as tp,
            tc.tile_pool(name="psacc", bufs=2, space="PSUM") as psacc,
            tc.tile_pool(name="psxp", bufs=4, space="PSUM") as psxp,
            tc.tile_pool(name="pstiny", bufs=1, space="PSUM") as pstiny,
        ):
            # ---- persistent SBUF state ----
            bt_res = pp.tile([128, NT * PAD], F16, tag="bt_res")
            zA = pp.tile([128, W], F32, tag="zA")
            zB = pp.tile([128, W], F32, tag="zB")
            zrA = pp.tile([128, W], F16, tag="zrA")
            zrB = pp.tile([128, W], F16, tag="zrB")
            # fp16 history rings: f_i at slot i%4 (doubles as next matmul
            # input), G_i at slot i%4
            fh16 = [pp.tile([128, W], F16, tag=f"fh{k}", name=f"fh{k}")
                    for k in range(4)]
            gh16 = [pp.tile([128, W], F16, tag=f"gh{k}", name=f"gh{k}")
                    for k in range(4)]
            gh6x = pp.tile([128, W], F16, tag="gh6x")  # G_6 (slot 2 aliases G_2)
            ident_sb = pp.tile([128, 128], F32, tag="ident")
            ones_col = pp.tile([128, 1], F32, tag="ones_col")
            ones_row = pp.tile([1, 128], F32, tag="ones_row")
            eps_sb = pp.tile([128, 1], F32, tag="eps")
            a_rep = pp.tile([128, 5], F32, tag="a_rep")
            # mesh-exchange state
            sendb = {6: pp.tile([128, 32], F32, tag="send6", name="send6"),
                     7: pp.tile([128, 32], F32, tag="send7", name="send7")}
            recvb = {6: pp.tile([128, 256], F32, tag="recv6", name="recv6"),
                     7: pp.tile([128, 256], F32, tag="recv7", name="recv7")}
            tokt = {6: pp.tile([128, 1], F32, tag="tok6", name="tok6"),
                    7: pp.tile([128, 1], F32, tag="tok7", name="tok7")}
            # Anderson small state (persistent single tiles)
            s5g = tp.tile([1, 32], F32, tag="s5g")
            haug = tp.tile([1, 4, 5], F32, tag="haug")
            gam = tp.tile([1, 4], F32, tag="gam")
            arow = tp.tile([1, 5], F32, tag="arow")
            dgrow = tp.tile([1, 3], F32, tag="dgrow")
            t1 = tp.tile([1, 4, 4], F32, tag="t1")
            t2 = tp.tile([1, 4, 4], F32, tag="t2")
            rowtmp = tp.tile([1, 5], F32, tag="rowtmp")
            lamt = tp.tile([1, 1], F32, tag="lamt")
            nc.vector.memset(lamt[:], LAM)
            zerot = tp.tile([1, 1], F32, tag="zerot")
            nc.vector.memset(zerot[:], 0.0)
            negone = pp.tile([128, 1], F32, tag="negone")
            nc.vector.memset(negone[:], -1.0)
            if apply_w:
                wrep_sb = pp.tile([128, 128], F32, tag="wrep")
                nc.sync.dma_start(out=wrep_sb[:], in_=wrep_in[:])
            if apply_b:
                brep_sb = pp.tile([128, 128], F32, tag="brep")
                nc.sync.dma_start(out=brep_sb[:], in_=brep_in[:])

            nc.gpsimd.sem_clear(rsem[6])
            nc.gpsimd.sem_clear(rsem[7])
            nc.gpsimd.sem_clear(lsem)

            # ---- loads (x0 first so the fp16 cast and the first matmuls
            # don't queue behind the 13 MB B transfer) ----
            nc.sync.dma_start(out=zA[:], in_=x0_in[:])
            nc.sync.dma_start(out=ident_sb[:], in_=ident_in[:])
            nc.vector.tensor_copy(out=zrA[:], in_=zA[:])  # fp32 -> fp16
            # split the 13 MB B load across the SP and ACT HWDGE queues
            for k in range(NT):
                eng = nc.sync if k % 2 == 0 else nc.scalar
                eng.dma_start(
                    out=bt_res[:, ts(k, PAD)], in_=bt_in[ts(k), :]
                )
            nc.vector.memset(ones_col[:], 1.0)
            nc.vector.memset(ones_row[:], 1.0)
            nc.vector.memset(eps_sb[:], LN_EPS)
            # NB: recvb is deliberately never written locally — remote writes
            # are the only writers, and reads are gated on the rsem wait.  A
            # local memset would be an unsynchronized multi-core race.
            for e in (6, 7):
                nc.vector.memset(sendb[e][:], 0.0)

            def mesh_allreduce_s5(step):
                """Global 5x5 Gram: sendb[step][0,0:25] (local window) summed
                across the 8 cores into s5g[:, 0:25] via 7 XOR-relative peer
                sends."""
                exch_count[step] += 1
                sb, rb, tk = sendb[step], recvb[step], tokt[step]
                preps = []
                for j in range(1, 8):
                    rdests = [None] * 8
                    rdests[j] = (0, j)
                    preps.append(nc.gpsimd.remote_dma_broadcast(
                        out_ap=rb[:, 32 * j:32 * (j + 1)],
                        in_ap=sb[:],
                        remote_sem=rsem[step],
                        local_sem=lsem,
                        rdests=rdests,
                    ))
                if last_trigger[0] is not None:
                    # SWDGE ring is FIFO: preps of this exchange must stay
                    # after the previous exchange's trigger
                    for p in preps:
                        add_dep_helper(
                            p.ins, last_trigger[0].ins,
                            reason="swdge ring order: prep after prev trigger",
                        )
                trig = nc.gpsimd.trigger_dma(count=None)
                if first_trigger[0] is None:
                    first_trigger[0] = trig
                last_trigger[0] = trig
                # tok = 0 * sendbuf[0,0]: reading sendbuf forces DVE stream
                # order scatter -> tok, so the attached rsem wait cannot
                # starve the trigger's dependency on the send data.  The
                # extra wait may exceed the 1-wait/instruction HW limit;
                # Bacc.generate_event_semaphores splits it at compile time.
                # col 24 = the (i,i) dot, the LAST scatter into this buffer
                tok_i = nc.vector.tensor_scalar_mul(
                    out=tk[0:1, :], in0=sb[0:1, 24:25], scalar1=zerot[:],
                )
                tok_fixups.append((tok_i, rsem[step], 14 * exch_count[step]))
                # s5g = local + tok (tok=0 carries the data-arrival dep)
                nc.vector.tensor_scalar(
                    out=s5g[:, 0:25], in0=sb[0:1, 0:25],
                    scalar1=tk[0:1, :], scalar2=None, op0=addop,
                )
                for j in range(1, 8):
                    nc.vector.tensor_tensor(
                        out=s5g[:, 0:25], in0=s5g[:, 0:25],
                        in1=rb[0:1, 32 * j:32 * j + 25], op=addop,
                    )

            zbuf, ybuf = zA, zB

            def zr_src(i):
                """fp16 matmul input holding z_i."""
                if i == 0:
                    return zrA
                if i == 1:
                    return zrB       # f_0
                if i == 2:
                    return zrA       # f_1
                if i <= 6:
                    return fh16[(i - 1) % 4]   # f_{i-1}
                return zrB           # z_7 from the Anderson combo

            def ln_cast_target(i):
                """Where LN output f_i gets cast to fp16 (None = nowhere)."""
                if i == 0:
                    return zrB
                if i == 1:
                    return zrA
                if 2 <= i <= 5:
                    return fh16[i % 4]   # history + next matmul input
                # f_6 -> fh16[2] would clobber f_2, still live for the
                # iteration-6 combo; it is cast after the combo instead
                return None

            for rep in range(repeat):
                for i in range(MAX_ITER):
                    win = _window(i)
                    last_rep = rep == repeat - 1
                    # ------- f(z): matmul + transpose + layernorm -------
                    if i == 0:
                        # k-outer in triples: consume each arriving B chunk
                        # for three accumulators at once so the PE chases the
                        # initial load (third slot uses the spare PSUM bank)
                        n_groups = [(0, 1, 2), (3, 4)]
                    else:
                        n_groups = [(n,) for n in range(NB)]
                    for grp in n_groups:
                        accs = {}
                        for gi, n in enumerate(grp):
                            spare = i == 0 and gi == 2
                            accs[n] = psacc.tile([128, 512], F32,
                                                 tag="acc0" if spare else "acc",
                                                 bufs=1 if spare else None,
                                                 name=f"acc_{rep}_{i}_{n}")
                        for k in range(NT):
                            for n in grp:
                                nc.tensor.matmul(
                                    accs[n][:],
                                    lhsT=zr_src(i)[:, ts(k)],
                                    rhs=bt_res[:, k * PAD + n * 512:k * PAD + (n + 1) * 512],
                                    start=(k == 0),
                                    stop=(k == NT - 1),
                                )
                        # consumption stays inside the group so at most two
                        # PSUM accumulators are ever live (psacc bufs=2)
                        for n in grp:
                            acc = accs[n]
                            xsb = xsbp.tile([128, 512], F32, tag="xsb")
                            nc.scalar.copy(out=xsb[:], in_=acc[:])
                            for jj in range(4):
                                j = n * 4 + jj
                                xp = psxp.tile([128, 128], F32, tag="xp")
                                nc.tensor.transpose(
                                    xp[:], in_=xsb[:, ts(jj)], identity=ident_sb[:]
                                )
                                bn6 = stp.tile([128, 6], F32, tag="bn6")
                                nc.vector.bn_stats(out=bn6[:], in_=xp[:])
                                mv = stp.tile([128, 2], F32, tag="mv")
                                nc.vector.bn_aggr(out=mv[:], in_=bn6[:])
                                rstd = stp.tile([128, 1], F32, tag="rstd")
                                nc.scalar.activation(
                                    out=rstd[:], in_=mv[:, 1:2], func=AF.Sqrt,
                                    bias=eps_sb[:], scale=1.0,
                                )
                                nc.vector.reciprocal(out=rstd[:], in_=rstd[:])
                                nmr = stp.tile([128, 1], F32, tag="nmr")
                                nc.vector.tensor_scalar(
                                    out=nmr[:], in0=mv[:, 0:1],
                                    scalar1=rstd[:], scalar2=negone[:],
                                    op0=mult, op1=mult,
                                )
                                nc.scalar.activation(
                                    out=ybuf[:, ts(j)], in_=xp[:],
                                    func=AF.Identity,
                                    bias=nmr[:], scale=rstd[:],
                                )
                                if apply_w:
                                    nc.vector.tensor_tensor(
                                        out=ybuf[:, ts(j)], in0=ybuf[:, ts(j)],
                                        in1=wrep_sb[:], op=mult,
                                    )
                                if apply_b:
                                    nc.vector.tensor_tensor(
                                        out=ybuf[:, ts(j)], in0=ybuf[:, ts(j)],
                                        in1=brep_sb[:], op=addop,
                                    )
                                tgt = ln_cast_target(i)
                                if tgt is not None:
                                    # f_i cast on ACT (DVE is the busier
                                    # engine); per-tile so the next
                                    # iteration's matmuls stay pipelined
                                    nc.scalar.activation(
                                        out=tgt[:, ts(j)], in_=ybuf[:, ts(j)],
                                        func=AF.Identity, bias=0.0, scale=1.0,
                                    )

                    # ------- history bookkeeping (iters 2..7 only) -------
                    if i >= 2 and debug < 2:
                        # fp16 home of G_i (also the dots' left operand)
                        if i <= 5:
                            g16_i = gh16[i % 4]
                        elif i == 6:
                            g16_i = gh6x
                        else:
                            g16_i = None   # G_7: per-chunk scratch cast
                        pairs = [a for a in win if a >= 2]
                        dcols = {a: stp.tile([128, NB], F32, tag="dcols",
                                             name=f"dcols_{rep}_{i}_{a}")
                                 for a in pairs}
                        # chunk-outer: each chunk's dot work starts as soon
                        # as its 4 LN tiles land; G is computed directly in
                        # fp16 (its only consumers are the fp16 dots)
                        for c in range(NB):
                            cs = ts(c, 512)
                            if g16_i is not None:
                                gc16 = g16_i[:, cs]
                            else:
                                g7s = xs16p.tile([128, 512], F16, tag="g7s")
                                gc16 = g7s[:]
                            nc.vector.tensor_tensor(
                                out=gc16, in0=ybuf[:, cs],
                                in1=zbuf[:, cs], op=sub,
                            )
                            for a in pairs:
                                if a == i:
                                    rhs16 = gc16
                                elif a == 6:
                                    rhs16 = gh6x[:, cs]
                                else:
                                    rhs16 = gh16[a % 4][:, cs]
                                scr = xs16p.tile([128, 512], F16, tag="sc16")
                                nc.vector.tensor_tensor(
                                    out=scr[:], in0=gc16, in1=rhs16, op=mult,
                                )
                                # free-dim reduce: 1/2 on ACT, 1/2 on DVE
                                # (DVE owns the multiplies already)
                                if (pairs.index(a) + c) % 2 == 0:
                                    nc.scalar.activation(
                                        out=scr[:], in_=scr[:],
                                        func=AF.Identity, bias=0.0, scale=1.0,
                                        accum_out=dcols[a][:, c:c + 1],
                                    )
                                else:
                                    nc.vector.tensor_reduce(
                                        out=dcols[a][:, c:c + 1], in_=scr[:],
                                        axis=mybir.AxisListType.X,
                                        op=addop,
                                    )
                        for a in pairs:
                            psd = pstiny.tile([1, 1], F32, tag="tinyps")
                            for c in range(NB):
                                nc.tensor.matmul(
                                    psd[:], lhsT=ones_col[:],
                                    rhs=dcols[a][:, c:c + 1],
                                    start=(c == 0), stop=(c == NB - 1),
                                )
                            dval = stp.tile([1, 1], F32, tag="dval")
                            nc.vector.tensor_copy(out=dval[:], in_=psd[:])
                            # scatter into the send buffers of steps 6 and 7
                            for step in (6, 7):
                                wv = _window(step)
                                if i in wv and a in wv:
                                    wi, wa = wv.index(i), wv.index(a)
                                    sb = sendb[step]
                                    nc.vector.tensor_copy(
                                        out=sb[0:1, wi * 5 + wa:wi * 5 + wa + 1],
                                        in_=dval[:],
                                    )
                                    if wi != wa:
                                        nc.vector.tensor_copy(
                                            out=sb[0:1, wa * 5 + wi:wa * 5 + wi + 1],
                                            in_=dval[:],
                                        )


                    # ------- Anderson mix (iterations 6 and 7) -------
                    if len(win) > 1 and i > 5 and debug < 1:
                        mesh_allreduce_s5(i)
                        s3 = s5g[:, 0:25].rearrange("p (a b) -> p a b", a=5)
                        # H = D S D^T, then + LAM on the diagonal
                        nc.vector.tensor_tensor(
                            out=t1[:], in0=s3[:, 1:5, 1:5], in1=s3[:, 1:5, 0:4], op=sub
                        )
                        nc.vector.tensor_tensor(
                            out=t2[:], in0=s3[:, 0:4, 1:5], in1=s3[:, 0:4, 0:4], op=sub
                        )
                        nc.vector.tensor_tensor(
                            out=haug[:, :, 0:4], in0=t1[:], in1=t2[:], op=sub
                        )
                        for jd in range(4):
                            nc.vector.tensor_tensor(
                                out=haug[:, jd, jd:jd + 1],
                                in0=haug[:, jd, jd:jd + 1], in1=lamt[:], op=addop,
                            )
                        # rhs_j = S[j+1, last] - S[j, last]
                        nc.vector.tensor_tensor(
                            out=haug[:, :, 4:5], in0=s3[:, 1:5, 4:5],
                            in1=s3[:, 0:4, 4:5], op=sub,
                        )
                        # unrolled Gaussian elimination (SPD + ridge: no pivoting)
                        for kk in range(3):
                            piv = stp.tile([1, 1], F32, tag="piv")
                            nc.vector.reciprocal(out=piv[:], in_=haug[:, kk, kk:kk + 1])
                            for r in range(kk + 1, 4):
                                m = stp.tile([1, 1], F32, tag="melim")
                                nc.vector.tensor_tensor(
                                    out=m[:], in0=haug[:, r, kk:kk + 1], in1=piv[:],
                                    op=mult,
                                )
                                nc.vector.tensor_scalar_mul(
                                    out=rowtmp[:, 0:5 - kk], in0=haug[:, kk, kk:5],
                                    scalar1=m[:],
                                )
                                nc.vector.tensor_tensor(
                                    out=haug[:, r, kk:5], in0=haug[:, r, kk:5],
                                    in1=rowtmp[:, 0:5 - kk], op=sub,
                                )
                        for kk in range(3, -1, -1):
                            accv = stp.tile([1, 1], F32, tag="accv")
                            nc.vector.tensor_copy(out=accv[:], in_=haug[:, kk, 4:5])
                            for jd in range(kk + 1, 4):
                                mm = stp.tile([1, 1], F32, tag="melim")
                                nc.vector.tensor_tensor(
                                    out=mm[:], in0=haug[:, kk, jd:jd + 1],
                                    in1=gam[:, jd:jd + 1], op=mult,
                                )
                                nc.vector.tensor_tensor(
                                    out=accv[:], in0=accv[:], in1=mm[:], op=sub
                                )
                            piv = stp.tile([1, 1], F32, tag="piv")
                            nc.vector.reciprocal(out=piv[:], in_=haug[:, kk, kk:kk + 1])
                            nc.vector.tensor_tensor(
                                out=gam[:, kk:kk + 1], in0=accv[:], in1=piv[:], op=mult
                            )
                        # z_next = sum_k a_k F_k with
                        # a = [0.5 g0, 0.5(g1-g0), 0.5(g2-g1), 0.5(g3-g2), 1-0.5 g3]
                        nc.scalar.activation(
                            out=arow[:, 0:1], in_=gam[:, 0:1], func=AF.Identity,
                            bias=0.0, scale=0.5,
                        )
                        nc.vector.tensor_tensor(
                            out=dgrow[:], in0=gam[:, 1:4], in1=gam[:, 0:3], op=sub
                        )
                        nc.scalar.activation(
                            out=arow[:, 1:4], in_=dgrow[:], func=AF.Identity,
                            bias=0.0, scale=0.5,
                        )
                        nc.scalar.activation(
                            out=arow[:, 4:5], in_=gam[:, 3:4], func=AF.Identity,
                            bias=1.0, scale=-0.5,
                        )
                        psa = pstiny.tile([128, 5], F32, tag="tinyps")
                        nc.tensor.matmul(
                            psa[:], lhsT=ones_row[:], rhs=arow[:], start=True, stop=True
                        )
                        nc.vector.tensor_copy(out=a_rep[:], in_=psa[:])
                        znew = zbuf  # z_i is dead once G_i exists; reuse buffer
                        f_slots = [fh16[a % 4] for a in win[:-1]]
                        # chunked: the next iteration's matmuls (or the output
                        # DMA) start as soon as each chunk completes.  History
                        # terms alternate ACT and DVE to balance the engines.
                        for c in range(NB):
                            cs = ts(c, 512)
                            nc.scalar.activation(
                                out=znew[:, cs], in_=ybuf[:, cs], func=AF.Identity,
                                bias=0.0, scale=a_rep[:, 4:5],
                            )
                            for kd in range(len(f_slots)):
                                tmpc = xsbp.tile([128, 512], F32, tag="sctmp")
                                if kd != 3:
                                    nc.scalar.activation(
                                        out=tmpc[:], in_=f_slots[kd][:, cs],
                                        func=AF.Identity, bias=0.0,
                                        scale=a_rep[:, kd:kd + 1],
                                    )
                                else:
                                    nc.vector.tensor_scalar_mul(
                                        out=tmpc[:], in0=f_slots[kd][:, cs],
                                        scalar1=a_rep[:, kd:kd + 1],
                                    )
                                nc.vector.tensor_tensor(
                                    out=znew[:, cs], in0=znew[:, cs],
                                    in1=tmpc[:], op=addop,
                                )
                            if i < MAX_ITER - 1:
                                nc.vector.tensor_copy(
                                    out=zrB[:, cs], in_=znew[:, cs]
                                )
                                # f_6 history cast on ACT, after this chunk's
                                # combo terms consumed f_2 from the same slot
                                nc.scalar.activation(
                                    out=fh16[2][:, cs], in_=ybuf[:, cs],
                                    func=AF.Identity, bias=0.0, scale=1.0,
                                )
                            elif last_rep:
                                # chunk-overlapped output DMA (strip padding)
                                for jj in range(4):
                                    j = c * 4 + jj
                                    rows = min(128, NPG - j * 128)
                                    if rows <= 0:
                                        break
                                    nc.sync.dma_start(
                                        out=zout[j * 128:j * 128 + rows, :],
                                        in_=znew[:rows, ts(j)],
                                    )
                        # zbuf keeps holding z_{i+1}; ybuf reusable for y_{i+1}
                    else:
                        if i == MAX_ITER - 1 and last_rep:
                            # debug path: dump ybuf (f_7) directly
                            for j in range(NT):
                                rows = min(128, NPG - j * 128)
                                if rows <= 0:
                                    break
                                nc.sync.dma_start(
                                    out=zout[j * 128:j * 128 + rows, :],
                                    in_=ybuf[:rows, ts(j)],
                                )
                        zbuf, ybuf = ybuf, zbuf

    # ---- post-scheduling fixups: waits the scheduling sim cannot satisfy ----
    if first_trigger[0] is not None:
        nc._bir_kernel_barrier_sem_replica_groups.append(set(range(N_CORES)))
        first_trigger[0].wait_op(
            nc._bir_kernel_barrier_sem, nc.bir_kernel_barrier_sem_inc,
            "sem-ge", check=False,
        )
    for tok_i, sem, thr in tok_fixups:
        tok_i.wait_op(sem, thr, "sem-ge", check=False)

    nc.compile()
    return nc


def _get_nc(apply_w, apply_b):
    key = (apply_w, apply_b, _DEBUG, _REPEAT)
    if key not in _NC_CACHE:
        _NC_CACHE[key] = _build_nc(apply_w, apply_b, debug=_DEBUG, repeat=_REPEAT)
    return _NC_CACHE[key]


def _prepare_inputs(x_init, norm_K_diag, sparse_values, edge_rows, edge_cols):
    """Host-side shard prep: dense per-graph B^T (fp16) + node-layout x0."""
    x_init = np.asarray(x_init, dtype=np.float32)
    nkd = np.asarray(norm_K_diag, dtype=np.float32).reshape(-1)
    vals = np.asarray(sparse_values, dtype=np.float32)
    rows = np.asarray(edge_rows)
    cols = np.asarray(edge_cols)

    g = rows // NPG
    r_loc = rows - g * NPG
    c_loc = cols - g * NPG
    # BT[g, k, m] = B_g[m, k]: accumulate edge values at (col, row)
    BT = np.zeros((N_GRAPHS, PAD, PAD), dtype=np.float32)
    np.add.at(BT, (g, c_loc, r_loc), vals)
    idx = np.arange(NPG)
    for gg in range(N_GRAPHS):
        BT[gg, idx, idx] += nkd[gg * NPG:(gg + 1) * NPG]
    BT16 = BT.astype(np.float16)

    ident = np.eye(128, dtype=np.float32)
    in_maps = []
    for gg in range(N_GRAPHS):
        xpad = np.zeros((PAD, D), dtype=np.float32)
        xpad[:NPG] = x_init[gg * NPG:(gg + 1) * NPG]
        # node-layout: node j*128+p  ->  x0[p, j*128:(j+1)*128]
        x0 = np.ascontiguousarray(
            xpad.reshape(NT, 128, D).transpose(1, 0, 2).reshape(128, NT * 128)
        )
        in_maps.append({"bt": BT16[gg], "x0": x0, "ident": ident})
    return in_maps


def kernel(x_init, norm_K_diag, sparse_values, ln_w, ln_b, edge_rows,
           edge_cols, batch, max_iter):
    assert int(max_iter) == MAX_ITER, f"kernel hardcodes max_iter={MAX_ITER}"
    ln_w = np.asarray(ln_w, dtype=np.float32)
    ln_b = np.asarray(ln_b, dtype=np.float32)
    apply_w = not np.all(ln_w == 1.0)
    apply_b = not np.all(ln_b == 0.0)

    in_maps = _prepare_inputs(
        x_init, norm_K_diag, sparse_values, edge_rows, edge_cols
    )
    if apply_w:
        wrep = np.ascontiguousarray(np.broadcast_to(ln_w, (128, 128)))
        for m in in_maps:
            m["wrep"] = wrep
    if apply_b:
        brep = np.ascontiguousarray(np.broadcast_to(ln_b, (128, 128)))
        for m in in_maps:
            m["brep"] = brep

    nc = _get_nc(apply_w, apply_b)
    res = run_bass_kernel_spmd(nc, in_maps, list(range(N_CORES)))
    out = np.concatenate(
        [res.results[gg]["zout"] for gg in range(N_GRAPHS)], axis=0
    )
    return out.astype(np.float32)
